# Initial kernel scaffold
#
import numpy as np

R_TOTAL = 262144
NS = 128
NF = 128
N_CORES = 8
R_CORE = R_TOTAL // N_CORES
TILE_P = 128

_CACHE = {}


def _build(r_core, unroll=2):
    import concourse.bass as bass
    import concourse.tile as tile
    from concourse import mybir
    from contextlib import ExitStack

    f32 = mybir.dt.float32
    Alu = mybir.AluOpType
    Act = mybir.ActivationFunctionType

    n_tiles = r_core // TILE_P
    P = TILE_P
    N = NS

    nc = bass.Bass("TRN2", target_bir_lowering=False)

    d_d = nc.dram_tensor("densities", [r_core, N, 1], f32, kind="ExternalInput").ap()
    c_d = nc.dram_tensor("colors", [r_core, N, 3], f32, kind="ExternalInput").ap()
    t_d = nc.dram_tensor("t_vals", [r_core, N], f32, kind="ExternalInput").ap()
    u_d = nc.dram_tensor("u", [r_core, NF], f32, kind="ExternalInput").ap()

    rgb_d = nc.dram_tensor("rgb", [r_core, 3], f32, kind="ExternalOutput").ap()
    dep_d = nc.dram_tensor("depth", [r_core], f32, kind="ExternalOutput").ap()
    acc_d = nc.dram_tensor("acc", [r_core], f32, kind="ExternalOutput").ap()
    w_d = nc.dram_tensor("weights", [r_core, N], f32, kind="ExternalOutput").ap()
    s_d = nc.dram_tensor("samples", [r_core, NF], f32, kind="ExternalOutput").ap()

    d_v = d_d.rearrange("(n p) k one -> n p (k one)", p=P)
    c_v = c_d.rearrange("(n p) k c -> n p (k c)", p=P)
    t_v = t_d.rearrange("(n p) k -> n p k", p=P)
    u_v = u_d.rearrange("(n p) k -> n p k", p=P)
    rgb_v = rgb_d.rearrange("(n p) c -> n p c", p=P)
    dep_v = dep_d.rearrange("(n p one) -> n p one", p=P, one=1)
    acc_v = acc_d.rearrange("(n p one) -> n p one", p=P, one=1)
    w_v = w_d.rearrange("(n p) k -> n p k", p=P)
    s_v = s_d.rearrange("(n p) k -> n p k", p=P)

    with tile.TileContext(nc) as tc:
        with ExitStack() as stk:
            pool = stk.enter_context(tc.tile_pool(name="work", bufs=2))

            def stage_load(pipe, i):
                dt = pipe.intermediate_tile([P, N], f32)
                tt = pipe.intermediate_tile([P, N], f32)
                ut = pipe.intermediate_tile([P, NF], f32)
                ct = pipe.intermediate_tile([P, N * 3], f32)
                nc.sync.dma_start(out=dt, in_=d_v[i])
                nc.sync.dma_start(out=tt, in_=t_v[i])
                nc.sync.dma_start(out=ut, in_=u_v[i])
                nc.sync.dma_start(out=ct, in_=c_v[i])
                return dt, tt, ut, ct

            def stage_compute(pipe, i, ins):
                dt, tt, ut, ct = ins
                wt = pipe.intermediate_tile([P, N], f32)
                st = pipe.intermediate_tile([P, NF], f32)
                sm5 = pipe.intermediate_tile([P, 5], f32)

                tmp = pool.tile([P, N], f32, tag="tmp")
                tmp2 = pool.tile([P, N], f32, tag="tmp2")
                di = pool.tile([P, N], f32, tag="di")
                nc.vector.tensor_tensor(out=di[:, 0:N-1], in0=tt[:, 1:N],
                                        in1=tt[:, 0:N-1], op=Alu.subtract)
                nc.vector.memset(di[:, N-1:N], 1e10)
                nc.vector.tensor_scalar(tmp, dt, 0.0, None, Alu.max)
                nc.vector.tensor_tensor(out=tmp, in0=tmp, in1=di, op=Alu.mult)
                av = pool.tile([P, N], f32, tag="av")
                nc.scalar.activation(av, tmp, Act.Exp, bias=0.0, scale=-1.0)
                al = pool.tile([P, N], f32, tag="al")
                nc.vector.tensor_scalar(al, av, -1.0, 1.0, Alu.mult, Alu.add)
                om = pool.tile([P, N], f32, tag="om")
                nc.vector.tensor_scalar(om, al, -1.0, 1.0, Alu.mult, Alu.add)
                nc.vector.tensor_scalar(om, om, 1e-10, None, Alu.add)
                sh = pool.tile([P, N], f32, tag="sh")
                nc.vector.memset(sh[:, 0:1], 1.0)
                nc.vector.tensor_copy(out=sh[:, 1:N], in_=om[:, 0:N-1])
                tr = pool.tile([P, N], f32, tag="tr")
                nc.vector.tensor_tensor_scan(tr, sh, sh, 1.0, Alu.mult, Alu.bypass)
                nc.vector.tensor_tensor(out=wt, in0=al, in1=tr, op=Alu.mult)

                cview = ct.rearrange("p (k c) -> p k c", c=3)
                for ch in range(3):
                    nc.vector.tensor_tensor_reduce(
                        out=tmp2, in0=wt, in1=cview[:, :, ch], scale=1.0,
                        scalar=0.0, op0=Alu.mult, op1=Alu.add,
                        accum_out=sm5[:, ch:ch+1])
                nc.vector.tensor_tensor_reduce(
                    out=tmp2, in0=wt, in1=tt, scale=1.0, scalar=0.0,
                    op0=Alu.mult, op1=Alu.add, accum_out=sm5[:, 3:4])
                nc.vector.tensor_reduce(out=sm5[:, 4:5], in_=wt,
                                        axis=mybir.AxisListType.C, op=Alu.add)

                wm = pool.tile([P, N - 2], f32, tag="wm")
                nc.vector.tensor_scalar(wm, wt[:, 1:N-1], 1e-5, None, Alu.add)
                wsum = pool.tile([P, 1], f32, tag="wsum")
                nc.vector.tensor_reduce(out=wsum, in_=wm,
                                        axis=mybir.AxisListType.C, op=Alu.add)
                pdf = pool.tile([P, N - 2], f32, tag="pdf")
                nc.vector.tensor_scalar(pdf, wm, wsum, None, Alu.divide)
                cf = pool.tile([P, N - 1], f32, tag="cf")
                nc.vector.memset(cf[:, 0:1], 0.0)
                nc.vector.tensor_tensor_scan(cf[:, 1:N-1], pdf, pdf, 0.0,
                                             Alu.add, Alu.bypass)

                dn = pool.tile([P, N - 2], f32, tag="dn")
                nc.vector.tensor_tensor(out=dn, in0=cf[:, 1:N-1],
                                        in1=cf[:, 0:N-2], op=Alu.subtract)
                g = pool.tile([P, N - 2], f32, tag="g")
                nc.vector.tensor_scalar(g, dn, 1e-5, None, Alu.is_lt)
                one_m_dn = pool.tile([P, N - 2], f32, tag="omdn")
                nc.vector.tensor_scalar(one_m_dn, dn, -1.0, 1.0, Alu.mult, Alu.add)
                dng = pool.tile([P, N - 2], f32, tag="dng")
                nc.vector.scalar_tensor_tensor(dng, g, 1.0, one_m_dn,
                                               Alu.mult, Alu.mult)
                nc.vector.tensor_tensor(out=dng, in0=dn, in1=dng, op=Alu.add)
                Sv = pool.tile([P, N - 1], f32, tag="Sv")
                nc.vector.tensor_tensor(out=Sv[:, 0:N-2], in0=tt[:, 1:N-1],
                                        in1=tt[:, 0:N-2], op=Alu.subtract)
                nc.vector.tensor_tensor(out=Sv[:, 0:N-2], in0=Sv[:, 0:N-2],
                                        in1=dng, op=Alu.divide)
                nc.vector.memset(Sv[:, N-2:N-1], 0.0)
                Av = pool.tile([P, N - 1], f32, tag="Av")
                nc.vector.tensor_tensor(out=Av, in0=cf, in1=Sv, op=Alu.mult)
                nc.vector.tensor_tensor(out=Av, in0=tt[:, 0:N-1], in1=Av,
                                        op=Alu.subtract)
                dA = pool.tile([P, N - 2], f32, tag="dA")
                nc.vector.tensor_tensor(out=dA, in0=Av[:, 1:N-1],
                                        in1=Av[:, 0:N-2], op=Alu.subtract)
                dS = pool.tile([P, N - 2], f32, tag="dS")
                nc.vector.tensor_tensor(out=dS, in0=Sv[:, 1:N-1],
                                        in1=Sv[:, 0:N-2], op=Alu.subtract)

                accA = pool.tile([P, NF], f32, tag="accA")
                nc.vector.tensor_scalar(accA, ut, 0.0, Av[:, 0:1], Alu.mult, Alu.add)
                accS = pool.tile([P, NF], f32, tag="accS")
                nc.gpsimd.tensor_scalar(accS, ut, 0.0, Sv[:, 0:1], Alu.mult, Alu.add)

                for j in range(1, N - 1):
                    mk = pool.tile([P, NF], f32, tag=f"mk{j % 4}")
                    nc.vector.tensor_scalar(mk, ut, cf[:, j:j+1], None, Alu.is_ge)
                    nc.vector.scalar_tensor_tensor(
                        accA, mk, dA[:, j-1:j], accA, Alu.mult, Alu.add)
                    nc.gpsimd.scalar_tensor_tensor(
                        accS, mk, dS[:, j-1:j], accS, Alu.mult, Alu.add)

                nc.vector.tensor_tensor(out=st, in0=ut, in1=accS, op=Alu.mult)
                nc.vector.tensor_tensor(out=st, in0=st, in1=accA, op=Alu.add)
                return wt, st, sm5

            def stage_store(pipe, i, outs):
                wt, st, sm5 = outs
                nc.sync.dma_start(out=w_v[i], in_=wt)
                nc.sync.dma_start(out=s_v[i], in_=st)
                nc.sync.dma_start(out=rgb_v[i], in_=sm5[:, 0:3])
                nc.sync.dma_start(out=dep_v[i], in_=sm5[:, 3:4])
                nc.sync.dma_start(out=acc_v[i], in_=sm5[:, 4:5])

            tc.For_i_pipelined(stk, [stage_load, stage_compute, stage_store],
                               0, n_tiles, 1, unroll=unroll)

    return nc


def _get_nc(r_core):
    if r_core not in _CACHE:
        _CACHE[r_core] = _build(r_core)
    return _CACHE[r_core]


def kernel(densities, colors, t_vals, u):
    from concourse.bass_utils import run_bass_kernel_spmd

    nc = _get_nc(R_CORE)
    core_ids = list(range(N_CORES))
    in_maps = []
    for ci in core_ids:
        lo, hi = ci * R_CORE, (ci + 1) * R_CORE
        in_maps.append({
            "densities": np.ascontiguousarray(densities[lo:hi]),
            "colors": np.ascontiguousarray(colors[lo:hi]),
            "t_vals": np.ascontiguousarray(t_vals[lo:hi]),
            "u": np.ascontiguousarray(u[lo:hi]),
        })
    res = run_bass_kernel_spmd(nc, in_maps, core_ids)
    rs = res.results
    rgb = np.concatenate([np.asarray(r["rgb"]) for r in rs], axis=0)
    depth = np.concatenate([np.asarray(r["depth"]) for r in rs], axis=0)
    acc = np.concatenate([np.asarray(r["acc"]) for r in rs], axis=0)
    weights = np.concatenate([np.asarray(r["weights"]) for r in rs], axis=0)
    samples = np.concatenate([np.asarray(r["samples"]) for r in rs], axis=0)
    return rgb, depth, acc, weights, samples


# revision 15
# speedup vs baseline: 1.3603x; 1.3603x over previous
import numpy as np

R_TOTAL = 262144
NS = 128
NF = 128
N_CORES = 8
R_CORE = R_TOTAL // N_CORES
TILE_P = 128

_CACHE = {}


def _build(r_core, unroll=2):
    import concourse.bass as bass
    import concourse.bacc as bacc
    import concourse.tile as tile
    from concourse import mybir
    from contextlib import ExitStack

    f32 = mybir.dt.float32
    Alu = mybir.AluOpType
    Act = mybir.ActivationFunctionType

    n_tiles = r_core // TILE_P
    P = TILE_P
    N = NS

    nc = bacc.Bacc("TRN2", target_bir_lowering=False)

    d_d = nc.dram_tensor("densities", [r_core, N, 1], f32, kind="ExternalInput").ap()
    c_d = nc.dram_tensor("colors", [r_core, N, 3], f32, kind="ExternalInput").ap()
    t_d = nc.dram_tensor("t_vals", [r_core, N], f32, kind="ExternalInput").ap()
    u_d = nc.dram_tensor("u", [r_core, NF], f32, kind="ExternalInput").ap()

    rgb_d = nc.dram_tensor("rgb", [r_core, 3], f32, kind="ExternalOutput").ap()
    dep_d = nc.dram_tensor("depth", [r_core], f32, kind="ExternalOutput").ap()
    acc_d = nc.dram_tensor("acc", [r_core], f32, kind="ExternalOutput").ap()
    w_d = nc.dram_tensor("weights", [r_core, N], f32, kind="ExternalOutput").ap()
    s_d = nc.dram_tensor("samples", [r_core, NF], f32, kind="ExternalOutput").ap()

    d_v = d_d.rearrange("(n p) k one -> n p (k one)", p=P)
    c_v = c_d.rearrange("(n p) k c -> n p (k c)", p=P)
    t_v = t_d.rearrange("(n p) k -> n p k", p=P)
    u_v = u_d.rearrange("(n p) k -> n p k", p=P)
    rgb_v = rgb_d.rearrange("(n p) c -> n p c", p=P)
    dep_v = dep_d.rearrange("(n p one) -> n p one", p=P, one=1)
    acc_v = acc_d.rearrange("(n p one) -> n p one", p=P, one=1)
    w_v = w_d.rearrange("(n p) k -> n p k", p=P)
    s_v = s_d.rearrange("(n p) k -> n p k", p=P)

    with tile.TileContext(nc) as tc:
        with ExitStack() as stk:
            pool = stk.enter_context(tc.tile_pool(name="work", bufs=2))

            def stage_load(pipe, i):
                dt = pipe.intermediate_tile([P, N], f32)
                tt = pipe.intermediate_tile([P, N], f32)
                ut = pipe.intermediate_tile([P, NF], f32)
                ct = pipe.intermediate_tile([P, N * 3], f32)
                nc.sync.dma_start(out=dt, in_=d_v[i])
                nc.sync.dma_start(out=tt, in_=t_v[i])
                nc.sync.dma_start(out=ut, in_=u_v[i])
                nc.sync.dma_start(out=ct, in_=c_v[i])
                return dt, tt, ut, ct

            def stage_compute(pipe, i, ins):
                dt, tt, ut, ct = ins
                wt = pipe.intermediate_tile([P, N], f32)
                st = pipe.intermediate_tile([P, NF], f32)
                sm5 = pipe.intermediate_tile([P, 5], f32)

                tmp = pool.tile([P, N], f32, tag="tmp")
                tmp2 = pool.tile([P, N], f32, tag="tmp2")
                di = pool.tile([P, N], f32, tag="di")
                nc.vector.tensor_tensor(out=di[:, 0:N-1], in0=tt[:, 1:N],
                                        in1=tt[:, 0:N-1], op=Alu.subtract)
                nc.vector.memset(di[:, N-1:N], 1e10)
                nc.vector.tensor_scalar(tmp, dt, 0.0, None, Alu.max)
                nc.vector.tensor_tensor(out=tmp, in0=tmp, in1=di, op=Alu.mult)
                av = pool.tile([P, N], f32, tag="av")
                nc.scalar.activation(av, tmp, Act.Exp, bias=0.0, scale=-1.0)
                al = pool.tile([P, N], f32, tag="al")
                nc.vector.tensor_scalar(al, av, -1.0, 1.0, Alu.mult, Alu.add)
                om = pool.tile([P, N], f32, tag="om")
                nc.vector.tensor_scalar(om, al, -1.0, 1.0, Alu.mult, Alu.add)
                nc.vector.tensor_scalar(om, om, 1e-10, None, Alu.add)
                sh = pool.tile([P, N], f32, tag="sh")
                nc.vector.memset(sh[:, 0:1], 1.0)
                nc.vector.tensor_copy(out=sh[:, 1:N], in_=om[:, 0:N-1])
                tr = pool.tile([P, N], f32, tag="tr")
                shv = sh.rearrange("p (b j) -> p b j", j=16)
                trv = tr.rearrange("p (b j) -> p b j", j=16)
                nc.vector.tensor_copy(out=trv[:, :, 0], in_=shv[:, :, 0])
                for jj in range(1, 16):
                    nc.vector.tensor_tensor(out=trv[:, :, jj], in0=trv[:, :, jj-1],
                                            in1=shv[:, :, jj], op=Alu.mult)
                off8 = pool.tile([P, 8], f32, tag="off8")
                sh8 = pool.tile([P, 8], f32, tag="sh8")
                nc.vector.memset(sh8[:, 0:1], 1.0)
                nc.vector.tensor_copy(out=sh8[:, 1:8], in_=trv[:, 0:7, 15])
                nc.vector.tensor_tensor_scan(off8, sh8, sh8, 1.0, Alu.mult, Alu.bypass)
                for b in range(8):
                    nc.vector.tensor_scalar(trv[:, b, :], trv[:, b, :],
                                            off8[:, b:b+1], None, Alu.mult)
                nc.vector.tensor_tensor(out=wt, in0=al, in1=tr, op=Alu.mult)

                cview = ct.rearrange("p (k c) -> p k c", c=3)
                for ch in range(3):
                    nc.vector.tensor_tensor(out=tmp2, in0=wt,
                                            in1=cview[:, :, ch], op=Alu.mult)
                    nc.vector.tensor_reduce(out=sm5[:, ch:ch+1], in_=tmp2,
                                            axis=mybir.AxisListType.X, op=Alu.add)
                nc.vector.tensor_tensor(out=tmp2, in0=wt, in1=tt, op=Alu.mult)
                nc.vector.tensor_reduce(out=sm5[:, 3:4], in_=tmp2,
                                        axis=mybir.AxisListType.X, op=Alu.add)
                nc.vector.tensor_reduce(out=sm5[:, 4:5], in_=wt,
                                        axis=mybir.AxisListType.X, op=Alu.add)

                wm = pool.tile([P, N - 2], f32, tag="wm")
                nc.vector.tensor_scalar(wm, wt[:, 1:N-1], 1e-5, None, Alu.add)
                wmp = pool.tile([P, N], f32, tag="wmp")
                nc.vector.tensor_copy(out=wmp[:, 0:N-2], in_=wm)
                nc.vector.memset(wmp[:, N-2:N], 0.0)
                wmpv = wmp.rearrange("p (b j) -> p b j", j=16)
                ws8 = pool.tile([P, 8], f32, tag="ws8")
                nc.vector.tensor_reduce(out=ws8, in_=wmpv,
                                        axis=mybir.AxisListType.X, op=Alu.add)
                wsum = pool.tile([P, 1], f32, tag="wsum")
                nc.vector.tensor_reduce(out=wsum, in_=ws8,
                                        axis=mybir.AxisListType.X, op=Alu.add)
                wrec = pool.tile([P, 1], f32, tag="wrec")
                nc.vector.reciprocal(wrec, wsum)
                pdf = pool.tile([P, N], f32, tag="pdf")
                nc.vector.tensor_scalar(pdf[:, 0:N-2], wm, wrec, None, Alu.mult)
                nc.vector.memset(pdf[:, N-2:N], 0.0)
                cs = pool.tile([P, N], f32, tag="cs")
                pdv = pdf.rearrange("p (b j) -> p b j", j=16)
                csv = cs.rearrange("p (b j) -> p b j", j=16)
                nc.vector.tensor_copy(out=csv[:, :, 0], in_=pdv[:, :, 0])
                for jj in range(1, 16):
                    nc.vector.tensor_tensor(out=csv[:, :, jj], in0=csv[:, :, jj-1],
                                            in1=pdv[:, :, jj], op=Alu.add)
                coff8 = pool.tile([P, 8], f32, tag="coff8")
                csh8 = pool.tile([P, 8], f32, tag="csh8")
                nc.vector.memset(csh8[:, 0:1], 0.0)
                nc.vector.tensor_copy(out=csh8[:, 1:8], in_=csv[:, 0:7, 15])
                nc.vector.tensor_tensor_scan(coff8, csh8, csh8, 0.0, Alu.add, Alu.bypass)
                for b in range(8):
                    nc.vector.tensor_scalar(csv[:, b, :], csv[:, b, :],
                                            coff8[:, b:b+1], None, Alu.add)
                cf = pool.tile([P, N - 1], f32, tag="cf")
                nc.vector.memset(cf[:, 0:1], 0.0)
                nc.vector.tensor_copy(out=cf[:, 1:N-1], in_=cs[:, 0:N-2])

                dn = pool.tile([P, N - 2], f32, tag="dn")
                nc.vector.tensor_tensor(out=dn, in0=cf[:, 1:N-1],
                                        in1=cf[:, 0:N-2], op=Alu.subtract)
                gg = pool.tile([P, N - 2], f32, tag="gg")
                nc.vector.tensor_scalar(gg, dn, 1e-5, None, Alu.is_lt)
                omd = pool.tile([P, N - 2], f32, tag="omd")
                nc.vector.tensor_scalar(omd, dn, -1.0, 1.0, Alu.mult, Alu.add)
                nc.vector.scalar_tensor_tensor(gg, gg, 1.0, omd, Alu.mult, Alu.mult)
                nc.vector.tensor_tensor(out=dn, in0=dn, in1=gg, op=Alu.add)
                nc.vector.reciprocal(dn, dn)
                Sv = pool.tile([P, N - 1], f32, tag="Sv")
                nc.vector.tensor_tensor(out=Sv[:, 0:N-2], in0=tt[:, 1:N-1],
                                        in1=tt[:, 0:N-2], op=Alu.subtract)
                nc.vector.tensor_tensor(out=Sv[:, 0:N-2], in0=Sv[:, 0:N-2],
                                        in1=dn, op=Alu.mult)
                nc.vector.memset(Sv[:, N-2:N-1], 0.0)
                dSh = pool.tile([P, N - 2], f32, tag="dSh")
                nc.vector.tensor_tensor(out=dSh, in0=Sv[:, 1:N-1],
                                        in1=Sv[:, 0:N-2], op=Alu.subtract)
                nc.vector.tensor_scalar(dSh, dSh, 0.5, None, Alu.mult)
                ncf = pool.tile([P, N - 1], f32, tag="ncf")
                nc.vector.tensor_scalar(ncf, cf, -1.0, None, Alu.mult)

                ac2a = pool.tile([P, 2 * NF], f32, tag="ac2a")
                nc.vector.tensor_scalar(ac2a[:, 0:NF], ut, 0.0, tt[:, 0:1],
                                        Alu.mult, Alu.add)
                nc.vector.memset(ac2a[:, NF:2*NF], 0.0)
                ac2b = pool.tile([P, 2 * NF], f32, tag="ac2b")
                nc.vector.memset(ac2b, 0.0)
                accSa = pool.tile([P, NF], f32, tag="accSa")
                nc.vector.tensor_scalar(accSa, ut, 0.0, Sv[:, 0:1], Alu.mult, Alu.add)
                nc.vector.tensor_scalar(accSa, accSa, 0.5, None, Alu.mult)
                accSb = pool.tile([P, NF], f32, tag="accSb")
                nc.vector.memset(accSb, 0.0)

                for j in range(1, N - 1):
                    mkv = pool.tile([P, 2 * NF], f32, tag=f"mkv{j % 4}")
                    _mkv_eng = nc.gpsimd if (j % 2 == 0) else nc.vector
                    _mkv_eng.tensor_scalar(mkv[:, 0:NF], ut, cf[:, j:j+1],
                                           tt[:, j:j+1], Alu.is_ge, Alu.mult)
                    _mkv_eng.tensor_scalar(mkv[:, NF:2*NF], ut, cf[:, j:j+1],
                                           cf[:, j:j+1], Alu.is_ge, Alu.mult)
                    sg = pool.tile([P, NF], f32, tag=f"sg{j % 4}")
                    nc.scalar.activation(sg, ut, Act.Sign, bias=ncf[:, j:j+1],
                                         scale=1.0)
                    ac2 = ac2a if j % 2 else ac2b
                    accS = accSa if j % 2 else accSb
                    nc.vector.tensor_tensor(out=ac2, in0=ac2, in1=mkv, op=Alu.max)
                    nc.vector.scalar_tensor_tensor(
                        accS, sg, dSh[:, j-1:j], accS, Alu.mult, Alu.add)

                nc.vector.tensor_tensor(out=ac2a, in0=ac2a, in1=ac2b, op=Alu.max)
                nc.vector.tensor_tensor(out=accSa, in0=accSa, in1=accSb, op=Alu.add)
                nc.vector.tensor_tensor(out=st, in0=ut, in1=ac2a[:, NF:2*NF],
                                        op=Alu.subtract)
                nc.vector.tensor_tensor(out=st, in0=st, in1=accSa, op=Alu.mult)
                nc.vector.tensor_tensor(out=st, in0=st, in1=ac2a[:, 0:NF], op=Alu.add)
                return wt, st, sm5

            def stage_store(pipe, i, outs):
                wt, st, sm5 = outs
                nc.sync.dma_start(out=w_v[i], in_=wt)
                nc.sync.dma_start(out=s_v[i], in_=st)
                nc.sync.dma_start(out=rgb_v[i], in_=sm5[:, 0:3])
                nc.sync.dma_start(out=dep_v[i], in_=sm5[:, 3:4])
                nc.sync.dma_start(out=acc_v[i], in_=sm5[:, 4:5])

            tc.For_i_pipelined([stage_load, stage_compute, stage_store],
                               0, n_tiles, 1, unroll=unroll)

    nc.compile()
    return nc


def _get_nc(r_core):
    if r_core not in _CACHE:
        _CACHE[r_core] = _build(r_core)
    return _CACHE[r_core]


def kernel(densities, colors, t_vals, u, _trace=False):
    from concourse.bass_utils import run_bass_kernel_spmd

    nc = _get_nc(R_CORE)
    core_ids = list(range(N_CORES))
    in_maps = []
    for ci in core_ids:
        lo, hi = ci * R_CORE, (ci + 1) * R_CORE
        in_maps.append({
            "densities": np.ascontiguousarray(densities[lo:hi]),
            "colors": np.ascontiguousarray(colors[lo:hi]),
            "t_vals": np.ascontiguousarray(t_vals[lo:hi]),
            "u": np.ascontiguousarray(u[lo:hi]),
        })
    res = run_bass_kernel_spmd(nc, in_maps, core_ids, trace=_trace)
    if _trace:
        print(f"HW exec time: {res.exec_time_ns} ns")
    rs = res.results
    rgb = np.concatenate([np.asarray(r["rgb"]) for r in rs], axis=0)
    depth = np.concatenate([np.asarray(r["depth"]) for r in rs], axis=0)
    acc = np.concatenate([np.asarray(r["acc"]) for r in rs], axis=0)
    weights = np.concatenate([np.asarray(r["weights"]) for r in rs], axis=0)
    samples = np.concatenate([np.asarray(r["samples"]) for r in rs], axis=0)
    return rgb, depth, acc, weights, samples
